# revision 40
# baseline (speedup 1.0000x reference)
"""CIN block kernel for Trainium2 (8 NeuronCores, data-parallel over batch).

Reference computation (per layer l, h0 = feat):
    out_l[b,k,d] = relu( sum_{i,j} W_l[k,i,j] * h_l[b,i,d] * feat[b,j,d] + b_l[k] )
    h_{l+1} = out_l[:, :K/2, :]   (split-half, except last layer)
    result  = concat([out0[:,128:], out1[:,128:], out2[:,:]], axis=1).sum(-1)

Mapping (per core, B_local=64, BD = B_local*D = 2048), mixed fp16/fp8:
    Tensors live as [channel, (b,d)] with (b,d) flattened on the free dim.
    The h-chain (layer-0 both halves, layer-1 k<128 half) runs fp16 so
    quantization error never propagates.

    Layer 0 exploits z-symmetry: z0[i,j] = feat_i*feat_j = z0[j,i], so the
    1024 (i,j) pairs fold to 528 with W0sym[k,i,j] = W0[k,i,j]+W0[k,j,i]
    (diag kept once) -> 5 partition-chunks instead of 8, exact math.
    featHsym (the i-row per chunk slot) streams from DRAM in t-major
    chunks, so z0 is a single all-SBUF f16 broadcast mul per tile (fast
    2x_1p DVE mode) and no PE row-replication matmuls are needed.

    Layer 1: kh0 (h-chain) is fp16 over all 32 j's (consumes z16 quads);
    kh1 runs the first kh1_f16 quads in fp16 (no z8, no convert - they
    cut the ACT convert load and improve accuracy) and the rest as fp8
    e4m3 1-pass DoubleRow (z8 = ACT convert of z16; r1 sits ~2.5x below
    the global max so its fp8 noise is invisible). The DR quads trail
    dr_lag quads behind their converts and fill PE holes while z16
    production paces the fb broadcast DMA stream.

    Layer 2: both halves fp8 2-pass DoubleRow (W = e4m3 hi + e5m2 lo
    residual; z8 e4m3). z8 is produced directly (f16 x f16 -> e4m3
    tensor_mul, no z16 stage) on Pool/DVE, or as z16 + ACT convert
    ("acvt"), per a static per-quad plan tuned for engine balance.

    Scheduling notes: every DMA serializes on the shared DMA engines, so
    the emission order of transfers is tuned to need-times (fb broadcasts
    dominate and pace both L1 halves); fb buffers are per-quad so fb(1)
    recycles each buffer as soon as L2(0)'s z mul has read it; d-reduces
    are deferred to flush points so they never head-of-line block the DVE
    queue; h1 drains on ACT, h2 on DVE; dummy fill matmuls bridge known
    PE idle seams so the tensor engine's p-state clock never drops; the
    L2(1) matmuls run kh-split so r2a's drains overlap r2b's matmuls.
"""

import os
import sys

import numpy as np

for _p in ("/opt/trn_rl_repo", "/root/.axon_site/_ro/trn_rl_repo"):
    if os.path.isdir(_p) and _p not in sys.path:
        sys.path.insert(0, _p)

import concourse.bacc as bacc
import concourse.bass as bass
import concourse.mybir as mybir
import concourse.tile as tile
from concourse.bass_utils import run_bass_kernel_spmd

F32 = mybir.dt.float32
F16 = mybir.dt.float16
F8E4 = mybir.dt.float8e4
F8E5 = mybir.dt.float8e5
DR = mybir.MatmulPerfMode.DoubleRow
RELU = mybir.ActivationFunctionType.Relu
AXX = mybir.AxisListType.X
ADD = mybir.AluOpType.add
MAX = mybir.AluOpType.max
MULT = mybir.AluOpType.mult

NCORES = 8
B, F0, D = 512, 32, 32
BL = B // NCORES          # 64 batch rows per core
BD = BL * D               # 2048 free positions per core
NT = 512                  # free-dim tile (one PSUM bank)
HB = 1024                 # half of BD
K = 256                   # channels per layer
H = 128                   # hidden rows fed to layers 1,2 (split-half of 256)
NC0 = 5                   # symmetric layer-0 partition-chunks (528 pairs)
NG = F0 // 2              # 16 DoubleRow groups (j-pairs) per layer
NQ = F0 // 4              # 8 quad tiles (4 j's) per (layer, half)

_CACHE = {}
LAST_RESULTS = None


def _sym_pack():
    """Orient each unordered (i,j) pair to a column j == p%32 and pack into
    NC0 chunks of 128 slots; returns i_of[c][p] (i row per slot, -1 = pad)."""
    cols = {j: [j] for j in range(F0)}          # diag first
    cnt = [1] * F0
    for a in range(F0):
        for b in range(a + 1, F0):
            # parity orientation keeps every column's count <= 17 (< 4*NC0)
            jj, io = (b, a) if (a + b) % 2 == 0 else (a, b)
            cols[jj].append(io)
            cnt[jj] += 1
    assert max(cnt) <= 4 * NC0, cnt
    i_of = -np.ones((NC0, 128), np.int64)
    for j in range(F0):
        for s, io in enumerate(cols[j]):
            c, m = divmod(s, 4)
            i_of[c, 32 * m + j] = io
    return i_of


def _build_program(
    gp_zq=((), ()),           # per-half L1 quads whose z16 runs on Pool
    cvt_l1=None,              # quad -> engine for L1 z16->z8 converts
    l2_plan=None,             # quad -> engine for L2 z8: gp|dve direct, acvt
    ps_bufs=8,
    z16_bufs=3,
    z8_bufs=7,
    dp_bufs=5,
    warmup_mms=8,
    w_dma_eng="scalar",
    out_dma_split=True,
    h2_dve=True,
    dr_lag=(2, 3),
    l2_dve_first=False,
    red_late=True,
    fhs_late=True,
    fill_q=(0, 0),
    fill_seam=(20, 20, 70),
    z0_first=False,
    l2_kh_split=True,
    kh1_f16=(2, 2),
    tail_dve=False,
):
    if cvt_l1 is None:
        cvt_l1 = ({q: "act" for q in range(8)},) * 2
    if l2_plan is None:
        l2_plan = ({0: "gp", 4: "gp", 1: "dve", 2: "acvt", 6: "dve",
                    7: "dve", 3: "acvt", 5: "acvt"},) * 2

    def cvt_plan(q, half):
        m = cvt_l1[half] if isinstance(cvt_l1, tuple) else cvt_l1
        return m.get(q, "act")

    def l2p(q, half):
        m = l2_plan[half] if isinstance(l2_plan, tuple) else l2_plan
        return m.get(q, "dve")

    nc = bacc.Bacc("TRN2", target_bir_lowering=False, debug=False)

    featT_d = nc.dram_tensor("featT16", [F0, BD], F16, kind="ExternalInput").ap()
    featR_d = nc.dram_tensor("featR", [128, BD], F16, kind="ExternalInput").ap()
    fHs_d = nc.dram_tensor("featHsym", [128, NC0 * BD], F16, kind="ExternalInput").ap()
    w0_d = nc.dram_tensor("w0sym", [128, NC0 * K], F16, kind="ExternalInput").ap()
    w1h_d = nc.dram_tensor("w1h16", [128, F0 * 128], F16, kind="ExternalInput").ap()
    w1r_d = nc.dram_tensor("w1r8", [128, NG * 2 * 128], F8E4, kind="ExternalInput").ap()
    w1r16_d = nc.dram_tensor("w1r16", [128, F0 * 128], F16, kind="ExternalInput").ap()
    w2hi_d = nc.dram_tensor("w2hi8", [128, NG * 2 * 2 * 128], F8E4, kind="ExternalInput").ap()
    w2lo_d = nc.dram_tensor("w2lo8", [128, NG * 2 * 2 * 128], F8E5, kind="ExternalInput").ap()
    b0_d = nc.dram_tensor("b0t", [128, 2], F32, kind="ExternalInput").ap()
    b1_d = nc.dram_tensor("b1t", [128, 2], F32, kind="ExternalInput").ap()
    b2_d = nc.dram_tensor("b2t", [128, 2], F32, kind="ExternalInput").ap()
    out_d = nc.dram_tensor("out", [512, BL], F32, kind="ExternalOutput").ap()

    with tile.TileContext(nc) as tc:
        with (
            tc.tile_pool(name="const", bufs=1) as const,
            tc.tile_pool(name="ps", bufs=ps_bufs, space="PSUM") as ps,
            tc.tile_pool(name="z16p", bufs=z16_bufs) as z16p,
            tc.tile_pool(name="z16gp", bufs=1) as z16gp,
            tc.tile_pool(name="z8p", bufs=z8_bufs) as z8p,
            tc.tile_pool(name="zp0", bufs=2) as zp0,
            tc.tile_pool(name="dp", bufs=dp_bufs) as dp,
        ):
            wt = const.tile([128, NT], F16, name="warm_sb")
            nc.vector.memset(wt, 0.0)
            if warmup_mms:
                wps = ps.tile([128, NT], F32, tag="ps", name="warm_ps")
                for _ in range(warmup_mms):
                    nc.tensor.matmul(wps, wt[:, :128], wt, start=True, stop=True)

            def fill(n, tag):
                # keep the PE p-state ramped through a known z-production
                # hole: n dummy 256-free matmuls (~107ns each at full clock)
                if not n:
                    return
                fps = ps.tile([128, NT], F32, tag="ps", name=f"fill_{tag}")
                for _ in range(n):
                    nc.tensor.matmul(
                        fps[:, :256], wt[:, :128], wt[:, :256],
                        start=True, stop=True,
                    )

            # ---- L0 constants first: L0 is the front of the schedule ----
            # DMA order is latency-critical: every transfer serializes on the
            # shared DMA engines, so order = need time.
            featR = const.tile([128, BD], F16)
            nc.sync.dma_start(featR[:, :HB], featR_d[:, :HB])
            fHs = const.tile([128, NC0 * BD], F16, name="fHs")
            tw = NC0 * NT
            nc.sync.dma_start(fHs[:, 0:tw], fHs_d[:, 0:tw])
            w0 = const.tile([128, NC0 * K], F16)
            nc.sync.dma_start(w0, w0_d)
            nc.sync.dma_start(fHs[:, tw : 2 * tw], fHs_d[:, tw : 2 * tw])
            w1h = const.tile([128, F0 * 128], F16)
            hw1 = F0 * 128 // 2
            if not fhs_late:
                nc.sync.dma_start(featR[:, HB:], featR_d[:, HB:])
                for t in (2, 3):
                    nc.sync.dma_start(
                        fHs[:, t * tw : (t + 1) * tw],
                        fHs_d[:, t * tw : (t + 1) * tw],
                    )
            b0 = const.tile([128, 2], F32)
            b1 = const.tile([128, 2], F32)
            b2 = const.tile([128, 2], F32)

            wq = getattr(nc, w_dma_eng)
            wq.dma_start(b0, b0_d)
            wq.dma_start(b1, b1_d)
            wq.dma_start(b2, b2_d)

            h1 = const.tile([128, BD], F16)
            h2 = const.tile([128, BD], F16)
            fb_grp = 4  # one buffer per quad; both halves recycle the buffer
            fbh = [
                const.tile([128, fb_grp * HB], F16, name=f"fbh{i}")
                for i in range(NQ)
            ]

            def fb_buf(half, j):
                return fbh[j // fb_grp]

            rall = const.tile([128, 4 * BL], F32)
            r0 = rall[:, 0 * BL : 1 * BL]
            r1 = rall[:, 1 * BL : 2 * BL]
            r2a = rall[:, 2 * BL : 3 * BL]
            r2b = rall[:, 3 * BL : 4 * BL]

            def emit_fb(half, js=tuple(range(F0))):
                hoff = half * HB
                for j in js:
                    dst = fb_buf(half, j)[:, (j % fb_grp) * HB : (j % fb_grp + 1) * HB]
                    nc.sync.dma_start(
                        dst,
                        featT_d[j : j + 1, hoff : hoff + HB].to_broadcast([128, HB]),
                    )

            def drain_h(o_ps, bias_ap, t, h_out, dve=False):
                dst = h_out[:, t * NT : (t + 1) * NT]
                if dve:
                    nc.vector.tensor_scalar(dst, o_ps, bias_ap, 0.0, op0=ADD, op1=MAX)
                else:
                    nc.scalar.activation(dst, o_ps, RELU, bias=bias_ap)

            pending_red = []

            def drain_r(o_ps, bias_ap, t, r_out, dve=False):
                dx = dp.tile([128, NT], F16, tag="d", name=f"d_{t}")
                if dve:
                    nc.vector.tensor_scalar(dx, o_ps, bias_ap, 0.0, op0=ADD, op1=MAX)
                else:
                    nc.scalar.activation(dx, o_ps, RELU, bias=bias_ap)
                pending_red.append((r_out, t, dx))

            def red_flush(k=99):
                # d-reduces are deferred to a point where their dx inputs are
                # already materialized, so they never head-of-line-block the
                # DVE queue in front of z production.
                while pending_red and k > 0:
                    k -= 1
                    r_out, t, dx = pending_red.pop(0)
                    nc.vector.reduce_sum(
                        r_out[:, t * (NT // D) : (t + 1) * (NT // D)],
                        dx.rearrange("p (b d) -> p b d", d=D),
                        axis=AXX,
                    )

            def emit_l0(half):
                # ------ Layer 0 (fp16, symmetric 528-pair contraction) ------
                for t in (2 * half, 2 * half + 1):
                    z0 = zp0.tile([128, NC0 * NT], F16, tag="z0")
                    nc.vector.tensor_mul(
                        z0.rearrange("p (c n) -> p c n", c=NC0),
                        fHs[:, t * tw : (t + 1) * tw].rearrange(
                            "p (c n) -> p c n", c=NC0
                        ),
                        featR[:, t * NT : (t + 1) * NT]
                        .rearrange("p (one n) -> p one n", one=1)
                        .to_broadcast([128, NC0, NT]),
                    )
                    o0 = [
                        ps.tile([128, NT], F32, tag="ps", name=f"o0_{t}_{kh}")
                        for kh in range(2)
                    ]
                    for c in range(NC0):
                        for kh in range(2):
                            nc.tensor.matmul(
                                o0[kh],
                                w0[:, c * K + kh * 128 : c * K + (kh + 1) * 128],
                                z0[:, c * NT : (c + 1) * NT],
                                start=(c == 0),
                                stop=(c == NC0 - 1),
                            )
                    drain_h(o0[0], b0[:, 0:1], t, h1)
                    drain_r(o0[1], b0[:, 1:2], t, r0)

            def emit_zmul(half, ht, q, gp=False):
                """single broadcast-AP mul producing a [128, 4*HB] z16 quad."""
                if gp:
                    z16 = z16gp.tile([128, 4 * HB], F16, tag="z16g", name="z16g")
                else:
                    z16 = z16p.tile([128, 4 * HB], F16, tag="z16", name="z16")
                fb = fb_buf(half, 4 * q)
                fs = (4 * q) % fb_grp * HB
                hb4 = ht.rearrange("p (one n) -> p one n", one=1).to_broadcast(
                    [128, 4, HB]
                )
                eng = nc.gpsimd if gp else nc.vector
                eng.tensor_mul(
                    z16.rearrange("p (x n) -> p x n", x=4),
                    hb4,
                    fb[:, fs : fs + 4 * HB].rearrange("p (x n) -> p x n", x=4),
                )
                return z16

            def emit_cvt(half, q, z16, z8m):
                z8 = z8p.tile([128, 4 * HB], F8E4, tag="z8", name=f"z8_{half}_{q}")
                ce = cvt_plan(q, half)
                if ce == "act":
                    nc.scalar.copy(z8, z16)
                elif ce == "gp":
                    nc.gpsimd.tensor_copy(z8, z16)
                else:
                    nc.vector.tensor_scalar_mul(z8, z16, 1.0)
                z8m[q] = z8

            def emit_zmul8(half, ht, q, gp=False):
                """direct fp8 e4m3 product (no z16 stage)."""
                z8 = z8p.tile([128, 4 * HB], F8E4, tag="z8", name=f"z8d_{half}_{q}")
                fb = fb_buf(half, 4 * q)
                fs = (4 * q) % fb_grp * HB
                hbx = ht.rearrange("p (one n) -> p one n", one=1).to_broadcast(
                    [128, 4, HB]
                )
                fbx = fb[:, fs : fs + 4 * HB].rearrange("p (x n) -> p x n", x=4)
                zx = z8.rearrange("p (x n) -> p x n", x=4)
                eng = nc.gpsimd if gp else nc.vector
                eng.tensor_mul(zx, hbx, fbx)
                return z8

            def dr_mm(o_u, w8, zv, a, u, g, glast, first):
                zu = zv[:, 2 * a : 2 * a + 2, u * NT : (u + 1) * NT]
                nc.tensor.matmul(
                    o_u,
                    w8.rearrange("p (two m) -> p two m", two=2),
                    zu,
                    start=first,
                    stop=glast,
                    perf_mode=DR,
                    skip_group_check=True,
                )

            def emit_l1_dr_quad(q, orr, z8m, first=False):
                zv = z8m[q].rearrange("p (four n) -> p four n", four=4)
                for a in range(2):
                    g = 2 * q + a
                    wv = w1r[:, g * 256 : (g + 1) * 256]
                    for u in range(2):
                        dr_mm(orr[u], wv, zv, a, u, g,
                              glast=(g == NG - 1), first=(first and a == 0))

            def emit_l1(half, z8m):
                lag = dr_lag[half] if isinstance(dr_lag, tuple) else dr_lag
                # z production + kh0 fp16 matmuls; converts inline; kh1's
                # fp8 DR quads trail dr_lag quads behind (their z8 converts
                # need to finish) and fill PE holes when z16 lags fb DMA.
                # GP z-quads are emitted first so the slow engine works ahead.
                hoff = half * HB
                ht = h1[:, hoff : hoff + HB]
                o = [
                    ps.tile([128, NT], F32, tag="ps", name=f"o1h_{half}_{u}")
                    for u in range(2)
                ]
                orr = [
                    ps.tile([128, NT], F32, tag="ps", name=f"o1r_{half}_{u}")
                    for u in range(2)
                ]
                nf16 = kh1_f16[half] if isinstance(kh1_f16, tuple) else kh1_f16
                zq = {}
                for q in range(NQ):
                    if q in gp_zq[half]:
                        zq[q] = emit_zmul(half, ht, q, gp=True)
                for q in range(NQ):
                    if q == 3:
                        red_flush(2)
                    z16 = zq.get(q)
                    if z16 is None:
                        z16 = emit_zmul(half, ht, q)
                    if q >= nf16:
                        emit_cvt(half, q, z16, z8m)
                    for jj in range(4):
                        j = 4 * q + jj
                        for u in range(2):
                            nc.tensor.matmul(
                                o[u],
                                w1h[:, j * 128 : (j + 1) * 128],
                                z16[:, jj * HB + u * NT : jj * HB + (u + 1) * NT],
                                start=(q == 0 and jj == 0),
                                stop=(q == NQ - 1 and jj == 3),
                            )
                            if q < nf16:
                                # kh1 fp16 prefix quad: no z8, no convert
                                nc.tensor.matmul(
                                    orr[u],
                                    w1r16[:, j * 128 : (j + 1) * 128],
                                    z16[:, jj * HB + u * NT : jj * HB + (u + 1) * NT],
                                    start=(q == 0 and jj == 0),
                                    stop=False,
                                    skip_group_check=True,
                                )
                    if q >= lag and q - lag >= nf16:
                        emit_l1_dr_quad(q - lag, orr, z8m, first=(q - lag == nf16 and nf16 == 0))
                    fq = fill_q[half] if isinstance(fill_q, tuple) else fill_q
                    if fq:
                        fill(fq, f"l1_{half}_{q}")
                if half == 1:
                    red_flush(4)  # half-0's r2 reduces, after our z quads
                for q in range(max(NQ - lag, nf16), NQ):
                    emit_l1_dr_quad(q, orr, z8m, first=(q == nf16 and nf16 == 0))
                return o, orr

            def l2_qorder(half):
                # dve-first: direct quads draw the earliest-released z8 pool
                # slots, so the scheduler can start them right at h2.
                dve = [q for q in range(NQ) if l2p(q, half) == "dve"]
                gp = [q for q in range(NQ) if l2p(q, half) == "gp"]
                rest = [q for q in range(NQ) if q not in dve + gp]
                if l2_dve_first:
                    return dve + gp + rest
                return gp + dve + rest

            L2_MM = [0, 1, 2, 6, 4, 7, 3, 5]  # approx z8 completion order

            def emit_l2_z(half, z8m):
                hoff = half * HB
                ht = h2[:, hoff : hoff + HB]
                for q in l2_qorder(half):
                    if l2p(q, half) == "acvt":
                        z16 = emit_zmul(half, ht, q)
                        z8 = z8p.tile(
                            [128, 4 * HB], F8E4, tag="z8", name=f"z8c_{half}_{q}"
                        )
                        nc.scalar.copy(z8, z16)
                        z8m[q] = z8
                    else:
                        z8m[q] = emit_zmul8(half, ht, q, gp=(l2p(q, half) == "gp"))

            def emit_l2_mms(half, z8m, o, khs=(0, 1), drain=None):
                # khs: which K-halves to emit this pass. With a single kh the
                # accumulation group closes as soon as the last z8 arrives, so
                # its drains overlap the other kh's matmuls (shorter tail).
                order = [q for q in L2_MM if q in z8m] + [
                    q for q in range(NQ) if q not in L2_MM and q in z8m
                ]
                for qi, q in enumerate(order):
                    zv = z8m[q].rearrange("p (four n) -> p four n", four=4)
                    first_q = qi == 0
                    last_q = qi == len(order) - 1
                    for a in range(2):
                        g = 2 * q + a
                        for kh in khs:
                            base = (g * 2 + kh) * 256
                            for u in range(2):
                                zu = zv[:, 2 * a : 2 * a + 2, u * NT : (u + 1) * NT]
                                nc.tensor.matmul(
                                    o[kh][u],
                                    w2hi[:, base : base + 256].rearrange(
                                        "p (two m) -> p two m", two=2
                                    ),
                                    zu,
                                    start=(first_q and a == 0), stop=False,
                                    perf_mode=DR, skip_group_check=True,
                                )
                                nc.tensor.matmul(
                                    o[kh][u],
                                    w2lo[:, base : base + 256].rearrange(
                                        "p (two m) -> p two m", two=2
                                    ),
                                    zu,
                                    start=False, stop=(last_q and a == 1),
                                    perf_mode=DR, skip_group_check=True,
                                )
                return o

            def emit_out(half):
                cs = slice(half * BL // 2, (half + 1) * BL // 2)
                nc.sync.dma_start(out_d[0:128, cs], r0[:, cs])
                nc.sync.dma_start(out_d[128:256, cs], r1[:, cs])
                nc.sync.dma_start(out_d[256:384, cs], r2a[:, cs])
                nc.sync.dma_start(out_d[384:512, cs], r2b[:, cs])

            # ---------------- emission schedule ----------------
            # fb(0) with the layer-1/2 weight loads interleaved at their
            # need-times (everything serializes on the shared DMA engines).
            # fHs t2/t3 arrive late on purpose: that pins L0(1) into the
            # L2(0)-z window, where the PE needs filler work.
            emit_fb(0, js=range(0, 4))
            nc.sync.dma_start(w1h[:, :hw1], w1h_d[:, :hw1])
            if gp_zq[0]:
                emit_fb(0, js=range(28, 32))   # gp quad 7: Pool works ahead
            emit_fb(0, js=range(4, 12))
            w1r = const.tile([128, NG * 2 * 128], F8E4)
            nc.sync.dma_start(w1r, w1r_d)
            nf16_max = max(kh1_f16) if isinstance(kh1_f16, tuple) else kh1_f16
            w1r16 = None
            if nf16_max:
                w1r16 = const.tile([128, nf16_max * 4 * 128], F16)
                nc.sync.dma_start(w1r16, w1r16_d[:, : nf16_max * 4 * 128])
            nc.sync.dma_start(w1h[:, hw1:], w1h_d[:, hw1:])
            emit_fb(0, js=range(12, 28))
            if not gp_zq[0]:
                emit_fb(0, js=range(28, 32))
            if fhs_late:
                nc.sync.dma_start(featR[:, HB:], featR_d[:, HB:])
                for t in (2, 3):
                    nc.sync.dma_start(
                        fHs[:, t * tw : (t + 1) * tw],
                        fHs_d[:, t * tw : (t + 1) * tw],
                    )
            w2hi = const.tile([128, NG * 2 * 2 * 128], F8E4)
            nc.sync.dma_start(w2hi, w2hi_d)
            w2lo = const.tile([128, NG * 2 * 2 * 128], F8E5)
            nc.sync.dma_start(w2lo, w2lo_d)

            # Half-0's L1 A-phase runs right after L0(0).
            emit_l0(0)
            z8m_1 = {}
            o1h, o1r = emit_l1(0, z8m_1)
            for u in range(2):
                drain_h(o1h[u], b1[:, 0:1], u, h2, dve=h2_dve)
            for u in range(2):
                drain_r(o1r[u], b1[:, 1:2], u, r1)
            fill(fill_seam[0], "seam0")
            # L0(1) next: its z0 muls sit ahead of L2(0)'s z8 muls on the
            # DVE queue and its matmuls fill the PE during L2(0) z spin-up.
            emit_l0(1)

            z8m2 = {}
            o2 = [
                [ps.tile([128, NT], F32, tag="ps", name=f"o2_0_{kh}_{u}")
                 for u in range(2)]
                for kh in range(2)
            ]
            emit_l2_z(0, z8m2)
            # fb(1) recycles the per-quad fbh buffers as L2(0)'s z muls
            # finish reading them (the tile graph serializes per buffer).
            emit_fb(1)
            if not red_late:
                red_flush()
            fill(fill_seam[1], "seam1")
            emit_l2_mms(0, z8m2, o2)
            red_flush()   # r1(0) + r0 t2/t3 reduces (needed by out(0) only)
            for u in range(2):
                drain_r(o2[0][u], b2[:, 0:1], u, r2a)
                drain_r(o2[1][u], b2[:, 1:2], u, r2b)

            # half 1 (emit_l1's q3 flush emits half-0's r2 reduces; out(0)
            # follows so it reads fully-reduced tiles)
            z8m_1 = {}
            o1h, o1r = emit_l1(1, z8m_1)
            if out_dma_split:
                emit_out(0)
            for u in range(2):
                drain_h(o1h[u], b1[:, 0:1], 2 + u, h2, dve=h2_dve)
            for u in range(2):
                drain_r(o1r[u], b1[:, 1:2], 2 + u, r1)
            z8m2 = {}
            o2 = [
                [ps.tile([128, NT], F32, tag="ps", name=f"o2_1_{kh}_{u}")
                 for u in range(2)]
                for kh in range(2)
            ]
            emit_l2_z(1, z8m2)
            if not red_late:
                red_flush()
            fill(fill_seam[2], "seam2")
            emit_l2_mms(1, z8m2, o2)
            red_flush()
            if out_dma_split:
                cs1 = slice(BL // 2, BL)
                nc.sync.dma_start(out_d[0:128, cs1], r0[:, cs1])
            for u in range(2):
                drain_r(o2[0][u], b2[:, 0:1], 2 + u, r2a)
                drain_r(o2[1][u], b2[:, 1:2], 2 + u, r2b)
            red_flush()
            if out_dma_split:
                cs1 = slice(BL // 2, BL)
                nc.sync.dma_start(out_d[128:256, cs1], r1[:, cs1])
                nc.sync.dma_start(out_d[256:384, cs1], r2a[:, cs1])
                nc.sync.dma_start(out_d[384:512, cs1], r2b[:, cs1])

            if not out_dma_split:
                nc.sync.dma_start(out_d[0:128, :], r0)
                nc.sync.dma_start(out_d[128:256, :], r1)
                nc.sync.dma_start(out_d[256:384, :], r2a)
                nc.sync.dma_start(out_d[384:512, :], r2b)

    nc.compile()
    return nc


def _host_prep(feat, W0, b0, W1, b1, W2, b2):
    """Rearrange full inputs into the per-core in_maps."""
    import ml_dtypes

    E4 = ml_dtypes.float8_e4m3fn
    E5 = ml_dtypes.float8_e5m2
    feat = np.ascontiguousarray(feat, dtype=np.float32)

    # symmetric-folded W0: slot (c, p) holds pair (i_of[c,p], p%32)
    i_of = _sym_pack()
    Wsym = W0.reshape(K, F0, F0) + W0.reshape(K, F0, F0).transpose(0, 2, 1)
    Wsym[:, np.arange(F0), np.arange(F0)] = W0[:, np.arange(F0), np.arange(F0)]
    w0sym = np.zeros((128, NC0 * K), np.float16)
    p_ = np.arange(128)
    for c in range(NC0):
        live = i_of[c] >= 0
        ii = np.where(live, i_of[c], 0)
        jj = p_ % F0
        w0sym[:, c * K : (c + 1) * K] = np.where(
            live[:, None], Wsym[:, ii, jj].T, 0.0
        ).astype(np.float16)

    # L1 kh0 (h-half) fp16: [i, j*128 + m] = W1[m, i, j]
    w1h16 = np.ascontiguousarray(
        W1[:128].transpose(1, 2, 0).reshape(H, F0 * 128)
    ).astype(np.float16)
    # L1 kh1 fp8 1-pass: [i, (g*2+pair)*128 + m] = q8(W1[128+m, i, 2g+pair])
    w1r = W1[128:].transpose(1, 2, 0)            # [i, j, m]
    w1r8 = np.clip(w1r, -240, 240).astype(E4).reshape(H, F0 * 128)
    # fp16 copy for the kh1 f16 prefix quads
    w1r16 = np.ascontiguousarray(w1r.reshape(H, F0 * 128)).astype(np.float16)

    # L2 both kh fp8 2-pass: [i, ((g*2+kh)*2+pair)*128 + m] = W2[kh*128+m, i, 2g+pair]
    w2 = W2.transpose(1, 2, 0).reshape(H, NG, 2, 2, 128)  # [i, g, pair, kh, m]
    w2 = np.ascontiguousarray(w2.transpose(0, 1, 3, 2, 4))  # [i, g, kh, pair, m]
    w2hi = np.clip(w2, -240, 240).astype(E4)
    w2lo = (w2 - w2hi.astype(np.float32)).astype(E5)
    w2hi8 = w2hi.reshape(H, NG * 2 * 2 * 128)
    w2lo8 = w2lo.reshape(H, NG * 2 * 2 * 128)

    b0t = np.ascontiguousarray(b0.reshape(2, 128).T).astype(np.float32)
    b1t = np.ascontiguousarray(b1.reshape(2, 128).T).astype(np.float32)
    b2t = np.ascontiguousarray(b2.reshape(2, 128).T).astype(np.float32)

    ii_all = np.where(i_of >= 0, i_of, (p_ % F0)[None, :])  # [NC0, 128]

    in_maps = []
    for c in range(NCORES):
        fc = feat[c * BL : (c + 1) * BL]                        # [64, 32, 32]
        featT = np.ascontiguousarray(fc.transpose(1, 0, 2)).reshape(F0, BD)
        featT = featT.astype(np.float16)
        featR = np.ascontiguousarray(featT[p_ % F0])            # [128, BD]
        # t-major featHsym: [p, t*(NC0*NT) + cc*NT + n] = featT[i_of[cc,p], t*NT+n]
        fHs = np.concatenate(
            [
                featT[ii_all[cc], t * NT : (t + 1) * NT]
                for t in range(BD // NT)
                for cc in range(NC0)
            ],
            axis=1,
        )                                                        # [128, NC0*BD]
        in_maps.append(
            {
                "featT16": featT,
                "featR": featR,
                "featHsym": np.ascontiguousarray(fHs),
                "w0sym": w0sym,
                "w1h16": w1h16,
                "w1r8": w1r8.view(np.uint8),
                "w1r16": w1r16,
                "w2hi8": w2hi8.view(np.uint8),
                "w2lo8": w2lo8.view(np.uint8),
                "b0t": b0t,
                "b1t": b1t,
                "b2t": b2t,
            }
        )
    return in_maps


def kernel(feat, W0, b0, W1, b1, W2, b2):
    global LAST_RESULTS
    if "nc" not in _CACHE:
        _CACHE["nc"] = _build_program()
    nc = _CACHE["nc"]
    in_maps = _host_prep(feat, W0, b0, W1, b1, W2, b2)
    res = run_bass_kernel_spmd(nc, in_maps, core_ids=list(range(NCORES)))
    LAST_RESULTS = res
    out = np.concatenate([res.results[c]["out"].T for c in range(NCORES)], axis=0)
    return np.ascontiguousarray(out, dtype=np.float32)


# revision 43
# speedup vs baseline: 1.0025x; 1.0025x over previous
"""CIN block kernel for Trainium2 (8 NeuronCores, data-parallel over batch).

Reference computation (per layer l, h0 = feat):
    out_l[b,k,d] = relu( sum_{i,j} W_l[k,i,j] * h_l[b,i,d] * feat[b,j,d] + b_l[k] )
    h_{l+1} = out_l[:, :K/2, :]   (split-half, except last layer)
    result  = concat([out0[:,128:], out1[:,128:], out2[:,:]], axis=1).sum(-1)

Mapping (per core, B_local=64, BD = B_local*D = 2048), mixed fp16/fp8:
    Tensors live as [channel, (b,d)] with (b,d) flattened on the free dim.
    The h-chain (layer-0 both halves, layer-1 k<128 half) runs fp16 so
    quantization error never propagates.

    Layer 0 exploits z-symmetry: z0[i,j] = feat_i*feat_j = z0[j,i], so the
    1024 (i,j) pairs fold to 528 with W0sym[k,i,j] = W0[k,i,j]+W0[k,j,i]
    (diag kept once) -> 5 partition-chunks instead of 8, exact math.
    featHsym (the i-row per chunk slot) streams from DRAM in t-major
    chunks, so z0 is a single all-SBUF f16 broadcast mul per tile (fast
    2x_1p DVE mode) and no PE row-replication matmuls are needed.

    Layer 1: kh0 (h-chain) is fp16 over all 32 j's (consumes z16 quads);
    kh1 runs the first kh1_f16 quads in fp16 (no z8, no convert - they
    cut the ACT convert load and improve accuracy) and the rest as fp8
    e4m3 1-pass DoubleRow (z8 = ACT convert of z16; r1 sits ~2.5x below
    the global max so its fp8 noise is invisible). The DR quads trail
    dr_lag quads behind their converts and fill PE holes while z16
    production paces the fb broadcast DMA stream.

    Layer 2: both halves fp8 2-pass DoubleRow (W = e4m3 hi + e5m2 lo
    residual; z8 e4m3). z8 is produced directly (f16 x f16 -> e4m3
    tensor_mul, no z16 stage) on Pool/DVE, or as z16 + ACT convert
    ("acvt"), per a static per-quad plan tuned for engine balance.

    Scheduling notes: every DMA serializes on the shared DMA engines, so
    the emission order of transfers is tuned to need-times (fb broadcasts
    dominate and pace both L1 halves); fb buffers are per-quad so fb(1)
    recycles each buffer as soon as L2(0)'s z mul has read it; d-reduces
    are deferred to flush points so they never head-of-line block the DVE
    queue; h1 drains on ACT, h2 on DVE; dummy fill matmuls bridge known
    PE idle seams so the tensor engine's p-state clock never drops; the
    L2(1) matmuls run kh-split so r2a's drains overlap r2b's matmuls.
"""

import os
import sys

import numpy as np

for _p in ("/opt/trn_rl_repo", "/root/.axon_site/_ro/trn_rl_repo"):
    if os.path.isdir(_p) and _p not in sys.path:
        sys.path.insert(0, _p)

import concourse.bacc as bacc
import concourse.bass as bass
import concourse.mybir as mybir
import concourse.tile as tile
from concourse.bass_utils import run_bass_kernel_spmd

F32 = mybir.dt.float32
F16 = mybir.dt.float16
F8E4 = mybir.dt.float8e4
F8E5 = mybir.dt.float8e5
DR = mybir.MatmulPerfMode.DoubleRow
RELU = mybir.ActivationFunctionType.Relu
AXX = mybir.AxisListType.X
ADD = mybir.AluOpType.add
MAX = mybir.AluOpType.max
MULT = mybir.AluOpType.mult

NCORES = 8
B, F0, D = 512, 32, 32
BL = B // NCORES          # 64 batch rows per core
BD = BL * D               # 2048 free positions per core
NT = 512                  # free-dim tile (one PSUM bank)
HB = 1024                 # half of BD
K = 256                   # channels per layer
H = 128                   # hidden rows fed to layers 1,2 (split-half of 256)
NC0 = 5                   # symmetric layer-0 partition-chunks (528 pairs)
NG = F0 // 2              # 16 DoubleRow groups (j-pairs) per layer
NQ = F0 // 4              # 8 quad tiles (4 j's) per (layer, half)

_CACHE = {}
LAST_RESULTS = None


def _sym_pack():
    """Orient each unordered (i,j) pair to a column j == p%32 and pack into
    NC0 chunks of 128 slots; returns i_of[c][p] (i row per slot, -1 = pad)."""
    cols = {j: [j] for j in range(F0)}          # diag first
    cnt = [1] * F0
    for a in range(F0):
        for b in range(a + 1, F0):
            # parity orientation keeps every column's count <= 17 (< 4*NC0)
            jj, io = (b, a) if (a + b) % 2 == 0 else (a, b)
            cols[jj].append(io)
            cnt[jj] += 1
    assert max(cnt) <= 4 * NC0, cnt
    i_of = -np.ones((NC0, 128), np.int64)
    for j in range(F0):
        for s, io in enumerate(cols[j]):
            c, m = divmod(s, 4)
            i_of[c, 32 * m + j] = io
    return i_of


def _build_program(
    gp_zq=((), ()),           # per-half L1 quads whose z16 runs on Pool
    cvt_l1=None,              # quad -> engine for L1 z16->z8 converts
    l2_plan=None,             # quad -> engine for L2 z8: gp|dve direct, acvt
    ps_bufs=8,
    z16_bufs=3,
    z8_bufs=7,
    dp_bufs=5,
    warmup_mms=8,
    w_dma_eng="scalar",
    out_dma_split=True,
    h2_dve=True,
    dr_lag=(2, 3),
    l2_dve_first=False,
    red_late=True,
    fhs_late=True,
    fill_q=(0, 0),
    fill_seam=(20, 20, 70),
    z0_first=False,
    l2_kh_split=True,
    kh1_f16=(2, 2),
    tail_dve=False,
):
    if cvt_l1 is None:
        cvt_l1 = ({q: "act" for q in range(8)},) * 2
    if l2_plan is None:
        l2_plan = ({0: "gp", 4: "gp", 1: "dve", 2: "acvt", 6: "dve",
                    7: "dve", 3: "acvt", 5: "f16"},) * 2

    def cvt_plan(q, half):
        m = cvt_l1[half] if isinstance(cvt_l1, tuple) else cvt_l1
        return m.get(q, "act")

    def l2p(q, half):
        m = l2_plan[half] if isinstance(l2_plan, tuple) else l2_plan
        return m.get(q, "dve")

    nc = bacc.Bacc("TRN2", target_bir_lowering=False, debug=False)

    featT_d = nc.dram_tensor("featT16", [F0, BD], F16, kind="ExternalInput").ap()
    featR_d = nc.dram_tensor("featR", [128, BD], F16, kind="ExternalInput").ap()
    fHs_d = nc.dram_tensor("featHsym", [128, NC0 * BD], F16, kind="ExternalInput").ap()
    w0_d = nc.dram_tensor("w0sym", [128, NC0 * K], F16, kind="ExternalInput").ap()
    w1h_d = nc.dram_tensor("w1h16", [128, F0 * 128], F16, kind="ExternalInput").ap()
    w1r_d = nc.dram_tensor("w1r8", [128, NG * 2 * 128], F8E4, kind="ExternalInput").ap()
    w1r16_d = nc.dram_tensor("w1r16", [128, F0 * 128], F16, kind="ExternalInput").ap()
    w2hi_d = nc.dram_tensor("w2hi8", [128, NG * 2 * 2 * 128], F8E4, kind="ExternalInput").ap()
    w2lo_d = nc.dram_tensor("w2lo8", [128, NG * 2 * 2 * 128], F8E5, kind="ExternalInput").ap()
    w2f16_d = nc.dram_tensor("w2f16", [128, F0 * K], F16, kind="ExternalInput").ap()
    b0_d = nc.dram_tensor("b0t", [128, 2], F32, kind="ExternalInput").ap()
    b1_d = nc.dram_tensor("b1t", [128, 2], F32, kind="ExternalInput").ap()
    b2_d = nc.dram_tensor("b2t", [128, 2], F32, kind="ExternalInput").ap()
    out_d = nc.dram_tensor("out", [512, BL], F32, kind="ExternalOutput").ap()

    with tile.TileContext(nc) as tc:
        with (
            tc.tile_pool(name="const", bufs=1) as const,
            tc.tile_pool(name="ps", bufs=ps_bufs, space="PSUM") as ps,
            tc.tile_pool(name="z16p", bufs=z16_bufs) as z16p,
            tc.tile_pool(name="z16gp", bufs=1) as z16gp,
            tc.tile_pool(name="z8p", bufs=z8_bufs) as z8p,
            tc.tile_pool(name="zp0", bufs=2) as zp0,
            tc.tile_pool(name="dp", bufs=dp_bufs) as dp,
        ):
            wt = const.tile([128, NT], F16, name="warm_sb")
            nc.vector.memset(wt, 0.0)
            if warmup_mms:
                wps = ps.tile([128, NT], F32, tag="ps", name="warm_ps")
                for _ in range(warmup_mms):
                    nc.tensor.matmul(wps, wt[:, :128], wt, start=True, stop=True)

            def fill(n, tag):
                # keep the PE p-state ramped through a known z-production
                # hole: n dummy 256-free matmuls (~107ns each at full clock)
                if not n:
                    return
                fps = ps.tile([128, NT], F32, tag="ps", name=f"fill_{tag}")
                for _ in range(n):
                    nc.tensor.matmul(
                        fps[:, :256], wt[:, :128], wt[:, :256],
                        start=True, stop=True,
                    )

            # ---- L0 constants first: L0 is the front of the schedule ----
            # DMA order is latency-critical: every transfer serializes on the
            # shared DMA engines, so order = need time.
            featR = const.tile([128, BD], F16)
            nc.sync.dma_start(featR[:, :HB], featR_d[:, :HB])
            fHs = const.tile([128, NC0 * BD], F16, name="fHs")
            tw = NC0 * NT
            nc.sync.dma_start(fHs[:, 0:tw], fHs_d[:, 0:tw])
            w0 = const.tile([128, NC0 * K], F16)
            nc.sync.dma_start(w0, w0_d)
            nc.sync.dma_start(fHs[:, tw : 2 * tw], fHs_d[:, tw : 2 * tw])
            w1h = const.tile([128, F0 * 128], F16)
            hw1 = F0 * 128 // 2
            if not fhs_late:
                nc.sync.dma_start(featR[:, HB:], featR_d[:, HB:])
                for t in (2, 3):
                    nc.sync.dma_start(
                        fHs[:, t * tw : (t + 1) * tw],
                        fHs_d[:, t * tw : (t + 1) * tw],
                    )
            b0 = const.tile([128, 2], F32)
            b1 = const.tile([128, 2], F32)
            b2 = const.tile([128, 2], F32)

            wq = getattr(nc, w_dma_eng)
            wq.dma_start(b0, b0_d)
            wq.dma_start(b1, b1_d)
            wq.dma_start(b2, b2_d)

            h1 = const.tile([128, BD], F16)
            h2 = const.tile([128, BD], F16)
            fb_grp = 4  # one buffer per quad; both halves recycle the buffer
            fbh = [
                const.tile([128, fb_grp * HB], F16, name=f"fbh{i}")
                for i in range(NQ)
            ]

            def fb_buf(half, j):
                return fbh[j // fb_grp]

            rall = const.tile([128, 4 * BL], F32)
            r0 = rall[:, 0 * BL : 1 * BL]
            r1 = rall[:, 1 * BL : 2 * BL]
            r2a = rall[:, 2 * BL : 3 * BL]
            r2b = rall[:, 3 * BL : 4 * BL]

            def emit_fb(half, js=tuple(range(F0))):
                hoff = half * HB
                for j in js:
                    dst = fb_buf(half, j)[:, (j % fb_grp) * HB : (j % fb_grp + 1) * HB]
                    nc.sync.dma_start(
                        dst,
                        featT_d[j : j + 1, hoff : hoff + HB].to_broadcast([128, HB]),
                    )

            def drain_h(o_ps, bias_ap, t, h_out, dve=False):
                dst = h_out[:, t * NT : (t + 1) * NT]
                if dve:
                    nc.vector.tensor_scalar(dst, o_ps, bias_ap, 0.0, op0=ADD, op1=MAX)
                else:
                    nc.scalar.activation(dst, o_ps, RELU, bias=bias_ap)

            pending_red = []

            def drain_r(o_ps, bias_ap, t, r_out, dve=False):
                dx = dp.tile([128, NT], F16, tag="d", name=f"d_{t}")
                if dve:
                    nc.vector.tensor_scalar(dx, o_ps, bias_ap, 0.0, op0=ADD, op1=MAX)
                else:
                    nc.scalar.activation(dx, o_ps, RELU, bias=bias_ap)
                pending_red.append((r_out, t, dx))

            def red_flush(k=99):
                # d-reduces are deferred to a point where their dx inputs are
                # already materialized, so they never head-of-line-block the
                # DVE queue in front of z production.
                while pending_red and k > 0:
                    k -= 1
                    r_out, t, dx = pending_red.pop(0)
                    nc.vector.reduce_sum(
                        r_out[:, t * (NT // D) : (t + 1) * (NT // D)],
                        dx.rearrange("p (b d) -> p b d", d=D),
                        axis=AXX,
                    )

            def emit_l0(half):
                # ------ Layer 0 (fp16, symmetric 528-pair contraction) ------
                for t in (2 * half, 2 * half + 1):
                    z0 = zp0.tile([128, NC0 * NT], F16, tag="z0")
                    nc.vector.tensor_mul(
                        z0.rearrange("p (c n) -> p c n", c=NC0),
                        fHs[:, t * tw : (t + 1) * tw].rearrange(
                            "p (c n) -> p c n", c=NC0
                        ),
                        featR[:, t * NT : (t + 1) * NT]
                        .rearrange("p (one n) -> p one n", one=1)
                        .to_broadcast([128, NC0, NT]),
                    )
                    o0 = [
                        ps.tile([128, NT], F32, tag="ps", name=f"o0_{t}_{kh}")
                        for kh in range(2)
                    ]
                    for c in range(NC0):
                        for kh in range(2):
                            nc.tensor.matmul(
                                o0[kh],
                                w0[:, c * K + kh * 128 : c * K + (kh + 1) * 128],
                                z0[:, c * NT : (c + 1) * NT],
                                start=(c == 0),
                                stop=(c == NC0 - 1),
                            )
                    drain_h(o0[0], b0[:, 0:1], t, h1)
                    drain_r(o0[1], b0[:, 1:2], t, r0)

            def emit_zmul(half, ht, q, gp=False):
                """single broadcast-AP mul producing a [128, 4*HB] z16 quad."""
                if gp:
                    z16 = z16gp.tile([128, 4 * HB], F16, tag="z16g", name="z16g")
                else:
                    z16 = z16p.tile([128, 4 * HB], F16, tag="z16", name="z16")
                fb = fb_buf(half, 4 * q)
                fs = (4 * q) % fb_grp * HB
                hb4 = ht.rearrange("p (one n) -> p one n", one=1).to_broadcast(
                    [128, 4, HB]
                )
                eng = nc.gpsimd if gp else nc.vector
                eng.tensor_mul(
                    z16.rearrange("p (x n) -> p x n", x=4),
                    hb4,
                    fb[:, fs : fs + 4 * HB].rearrange("p (x n) -> p x n", x=4),
                )
                return z16

            def emit_cvt(half, q, z16, z8m):
                z8 = z8p.tile([128, 4 * HB], F8E4, tag="z8", name=f"z8_{half}_{q}")
                ce = cvt_plan(q, half)
                if ce == "act":
                    nc.scalar.copy(z8, z16)
                elif ce == "gp":
                    nc.gpsimd.tensor_copy(z8, z16)
                else:
                    nc.vector.tensor_scalar_mul(z8, z16, 1.0)
                z8m[q] = z8

            def emit_zmul8(half, ht, q, gp=False):
                """direct fp8 e4m3 product (no z16 stage)."""
                z8 = z8p.tile([128, 4 * HB], F8E4, tag="z8", name=f"z8d_{half}_{q}")
                fb = fb_buf(half, 4 * q)
                fs = (4 * q) % fb_grp * HB
                hbx = ht.rearrange("p (one n) -> p one n", one=1).to_broadcast(
                    [128, 4, HB]
                )
                fbx = fb[:, fs : fs + 4 * HB].rearrange("p (x n) -> p x n", x=4)
                zx = z8.rearrange("p (x n) -> p x n", x=4)
                eng = nc.gpsimd if gp else nc.vector
                eng.tensor_mul(zx, hbx, fbx)
                return z8

            def dr_mm(o_u, w8, zv, a, u, g, glast, first):
                zu = zv[:, 2 * a : 2 * a + 2, u * NT : (u + 1) * NT]
                nc.tensor.matmul(
                    o_u,
                    w8.rearrange("p (two m) -> p two m", two=2),
                    zu,
                    start=first,
                    stop=glast,
                    perf_mode=DR,
                    skip_group_check=True,
                )

            def emit_l1_dr_quad(q, orr, z8m, first=False):
                zv = z8m[q].rearrange("p (four n) -> p four n", four=4)
                for a in range(2):
                    g = 2 * q + a
                    wv = w1r[:, g * 256 : (g + 1) * 256]
                    for u in range(2):
                        dr_mm(orr[u], wv, zv, a, u, g,
                              glast=(g == NG - 1), first=(first and a == 0))

            def emit_l1(half, z8m):
                lag = dr_lag[half] if isinstance(dr_lag, tuple) else dr_lag
                # z production + kh0 fp16 matmuls; converts inline; kh1's
                # fp8 DR quads trail dr_lag quads behind (their z8 converts
                # need to finish) and fill PE holes when z16 lags fb DMA.
                # GP z-quads are emitted first so the slow engine works ahead.
                hoff = half * HB
                ht = h1[:, hoff : hoff + HB]
                o = [
                    ps.tile([128, NT], F32, tag="ps", name=f"o1h_{half}_{u}")
                    for u in range(2)
                ]
                orr = [
                    ps.tile([128, NT], F32, tag="ps", name=f"o1r_{half}_{u}")
                    for u in range(2)
                ]
                nf16 = kh1_f16[half] if isinstance(kh1_f16, tuple) else kh1_f16
                zq = {}
                for q in range(NQ):
                    if q in gp_zq[half]:
                        zq[q] = emit_zmul(half, ht, q, gp=True)
                for q in range(NQ):
                    if q == 3:
                        red_flush(2)
                    z16 = zq.get(q)
                    if z16 is None:
                        z16 = emit_zmul(half, ht, q)
                    if q >= nf16:
                        emit_cvt(half, q, z16, z8m)
                    for jj in range(4):
                        j = 4 * q + jj
                        for u in range(2):
                            nc.tensor.matmul(
                                o[u],
                                w1h[:, j * 128 : (j + 1) * 128],
                                z16[:, jj * HB + u * NT : jj * HB + (u + 1) * NT],
                                start=(q == 0 and jj == 0),
                                stop=(q == NQ - 1 and jj == 3),
                            )
                            if q < nf16:
                                # kh1 fp16 prefix quad: no z8, no convert
                                nc.tensor.matmul(
                                    orr[u],
                                    w1r16[:, j * 128 : (j + 1) * 128],
                                    z16[:, jj * HB + u * NT : jj * HB + (u + 1) * NT],
                                    start=(q == 0 and jj == 0),
                                    stop=False,
                                    skip_group_check=True,
                                )
                    if q >= lag and q - lag >= nf16:
                        emit_l1_dr_quad(q - lag, orr, z8m, first=(q - lag == nf16 and nf16 == 0))
                    fq = fill_q[half] if isinstance(fill_q, tuple) else fill_q
                    if fq:
                        fill(fq, f"l1_{half}_{q}")
                if half == 1:
                    red_flush(4)  # half-0's r2 reduces, after our z quads
                for q in range(max(NQ - lag, nf16), NQ):
                    emit_l1_dr_quad(q, orr, z8m, first=(q == nf16 and nf16 == 0))
                return o, orr

            def l2_qorder(half):
                # dve-first: direct quads draw the earliest-released z8 pool
                # slots, so the scheduler can start them right at h2.
                dve = [q for q in range(NQ) if l2p(q, half) == "dve"]
                gp = [q for q in range(NQ) if l2p(q, half) == "gp"]
                f16 = [q for q in range(NQ) if l2p(q, half) == "f16"]
                rest = [q for q in range(NQ) if q not in dve + gp + f16]
                if l2_dve_first:
                    return dve + gp + rest + f16
                return gp + dve + rest + f16

            L2_MM = [0, 1, 2, 6, 4, 7, 3, 5]  # approx z8 completion order

            def emit_l2_z(half, z8m, z16m):
                hoff = half * HB
                ht = h2[:, hoff : hoff + HB]
                for q in l2_qorder(half):
                    p = l2p(q, half)
                    if p == "f16":
                        z16m[q] = emit_zmul(half, ht, q)
                    elif p == "acvt":
                        z16 = emit_zmul(half, ht, q)
                        z8 = z8p.tile(
                            [128, 4 * HB], F8E4, tag="z8", name=f"z8c_{half}_{q}"
                        )
                        nc.scalar.copy(z8, z16)
                        z8m[q] = z8
                    else:
                        z8m[q] = emit_zmul8(half, ht, q, gp=(p == "gp"))

            def emit_l2_mms(half, z8m, z16m, o, khs=(0, 1)):
                # khs: which K-halves to emit this pass. With a single kh the
                # accumulation group closes as soon as the last z8 arrives, so
                # its drains overlap the other kh's matmuls (shorter tail).
                # f16 quads (z16m) run last: their z16 needs no convert, so
                # they shorten the tail's z8-production chain.
                order = [q for q in L2_MM if q in z8m] + [
                    q for q in range(NQ) if q not in L2_MM and q in z8m
                ] + [q for q in range(NQ) if q in z16m]
                for qi, q in enumerate(order):
                    first_q = qi == 0
                    last_q = qi == len(order) - 1
                    if q in z16m:
                        z16 = z16m[q]
                        si = l2f16q.index(q)
                        for jj in range(4):
                            jc = 4 * si + jj
                            for kh in khs:
                                ws = w2f16[:, jc * K + kh * 128 : jc * K + (kh + 1) * 128]
                                for u in range(2):
                                    nc.tensor.matmul(
                                        o[kh][u],
                                        ws,
                                        z16[:, jj * HB + u * NT : jj * HB + (u + 1) * NT],
                                        start=(first_q and jj == 0),
                                        stop=(last_q and jj == 3),
                                        skip_group_check=True,
                                    )
                        continue
                    zv = z8m[q].rearrange("p (four n) -> p four n", four=4)
                    for a in range(2):
                        g = 2 * q + a
                        for kh in khs:
                            base = (g * 2 + kh) * 256
                            for u in range(2):
                                zu = zv[:, 2 * a : 2 * a + 2, u * NT : (u + 1) * NT]
                                nc.tensor.matmul(
                                    o[kh][u],
                                    w2hi[:, base : base + 256].rearrange(
                                        "p (two m) -> p two m", two=2
                                    ),
                                    zu,
                                    start=(first_q and a == 0), stop=False,
                                    perf_mode=DR, skip_group_check=True,
                                )
                                nc.tensor.matmul(
                                    o[kh][u],
                                    w2lo[:, base : base + 256].rearrange(
                                        "p (two m) -> p two m", two=2
                                    ),
                                    zu,
                                    start=False, stop=(last_q and a == 1),
                                    perf_mode=DR, skip_group_check=True,
                                )
                return o

            def emit_out(half):
                cs = slice(half * BL // 2, (half + 1) * BL // 2)
                nc.sync.dma_start(out_d[0:128, cs], r0[:, cs])
                nc.sync.dma_start(out_d[128:256, cs], r1[:, cs])
                nc.sync.dma_start(out_d[256:384, cs], r2a[:, cs])
                nc.sync.dma_start(out_d[384:512, cs], r2b[:, cs])

            # ---------------- emission schedule ----------------
            # fb(0) with the layer-1/2 weight loads interleaved at their
            # need-times (everything serializes on the shared DMA engines).
            # fHs t2/t3 arrive late on purpose: that pins L0(1) into the
            # L2(0)-z window, where the PE needs filler work.
            emit_fb(0, js=range(0, 4))
            nc.sync.dma_start(w1h[:, :hw1], w1h_d[:, :hw1])
            if gp_zq[0]:
                emit_fb(0, js=range(28, 32))   # gp quad 7: Pool works ahead
            emit_fb(0, js=range(4, 12))
            w1r = const.tile([128, NG * 2 * 128], F8E4)
            nc.sync.dma_start(w1r, w1r_d)
            nf16_max = max(kh1_f16) if isinstance(kh1_f16, tuple) else kh1_f16
            w1r16 = None
            if nf16_max:
                w1r16 = const.tile([128, nf16_max * 4 * 128], F16)
                nc.sync.dma_start(w1r16, w1r16_d[:, : nf16_max * 4 * 128])
            nc.sync.dma_start(w1h[:, hw1:], w1h_d[:, hw1:])
            emit_fb(0, js=range(12, 28))
            if not gp_zq[0]:
                emit_fb(0, js=range(28, 32))
            if fhs_late:
                nc.sync.dma_start(featR[:, HB:], featR_d[:, HB:])
                for t in (2, 3):
                    nc.sync.dma_start(
                        fHs[:, t * tw : (t + 1) * tw],
                        fHs_d[:, t * tw : (t + 1) * tw],
                    )
            w2hi = const.tile([128, NG * 2 * 2 * 128], F8E4)
            nc.sync.dma_start(w2hi, w2hi_d)
            w2lo = const.tile([128, NG * 2 * 2 * 128], F8E5)
            nc.sync.dma_start(w2lo, w2lo_d)
            l2f16q = sorted({q for h in (0, 1) for q in range(NQ)
                             if l2p(q, h) == "f16"})
            w2f16 = None
            if l2f16q:
                w2f16 = const.tile([128, len(l2f16q) * 4 * K], F16)
                for si, q in enumerate(l2f16q):
                    nc.sync.dma_start(
                        w2f16[:, si * 4 * K : (si + 1) * 4 * K],
                        w2f16_d[:, q * 4 * K : (q + 1) * 4 * K],
                    )

            # Half-0's L1 A-phase runs right after L0(0).
            emit_l0(0)
            z8m_1 = {}
            o1h, o1r = emit_l1(0, z8m_1)
            for u in range(2):
                drain_h(o1h[u], b1[:, 0:1], u, h2, dve=h2_dve)
            for u in range(2):
                drain_r(o1r[u], b1[:, 1:2], u, r1)
            fill(fill_seam[0], "seam0")
            # L0(1) next: its z0 muls sit ahead of L2(0)'s z8 muls on the
            # DVE queue and its matmuls fill the PE during L2(0) z spin-up.
            emit_l0(1)

            z8m2 = {}
            z16m2 = {}
            o2 = [
                [ps.tile([128, NT], F32, tag="ps", name=f"o2_0_{kh}_{u}")
                 for u in range(2)]
                for kh in range(2)
            ]
            emit_l2_z(0, z8m2, z16m2)
            # fb(1) recycles the per-quad fbh buffers as L2(0)'s z muls
            # finish reading them (the tile graph serializes per buffer).
            emit_fb(1)
            if not red_late:
                red_flush()
            fill(fill_seam[1], "seam1")
            emit_l2_mms(0, z8m2, z16m2, o2)
            red_flush()   # r1(0) + r0 t2/t3 reduces (needed by out(0) only)
            for u in range(2):
                drain_r(o2[0][u], b2[:, 0:1], u, r2a)
                drain_r(o2[1][u], b2[:, 1:2], u, r2b)

            # half 1 (emit_l1's q3 flush emits half-0's r2 reduces; out(0)
            # follows so it reads fully-reduced tiles)
            z8m_1 = {}
            o1h, o1r = emit_l1(1, z8m_1)
            if out_dma_split:
                emit_out(0)
            for u in range(2):
                drain_h(o1h[u], b1[:, 0:1], 2 + u, h2, dve=h2_dve)
            for u in range(2):
                drain_r(o1r[u], b1[:, 1:2], 2 + u, r1)
            z8m2 = {}
            z16m2 = {}
            o2 = [
                [ps.tile([128, NT], F32, tag="ps", name=f"o2_1_{kh}_{u}")
                 for u in range(2)]
                for kh in range(2)
            ]
            emit_l2_z(1, z8m2, z16m2)
            if not red_late:
                red_flush()
            fill(fill_seam[2], "seam2")
            emit_l2_mms(1, z8m2, z16m2, o2)
            red_flush()
            if out_dma_split:
                cs1 = slice(BL // 2, BL)
                nc.sync.dma_start(out_d[0:128, cs1], r0[:, cs1])
            for u in range(2):
                drain_r(o2[0][u], b2[:, 0:1], 2 + u, r2a)
                drain_r(o2[1][u], b2[:, 1:2], 2 + u, r2b)
            red_flush()
            if out_dma_split:
                cs1 = slice(BL // 2, BL)
                nc.sync.dma_start(out_d[128:256, cs1], r1[:, cs1])
                nc.sync.dma_start(out_d[256:384, cs1], r2a[:, cs1])
                nc.sync.dma_start(out_d[384:512, cs1], r2b[:, cs1])

            if not out_dma_split:
                nc.sync.dma_start(out_d[0:128, :], r0)
                nc.sync.dma_start(out_d[128:256, :], r1)
                nc.sync.dma_start(out_d[256:384, :], r2a)
                nc.sync.dma_start(out_d[384:512, :], r2b)

    nc.compile()
    return nc


def _host_prep(feat, W0, b0, W1, b1, W2, b2):
    """Rearrange full inputs into the per-core in_maps."""
    import ml_dtypes

    E4 = ml_dtypes.float8_e4m3fn
    E5 = ml_dtypes.float8_e5m2
    feat = np.ascontiguousarray(feat, dtype=np.float32)

    # symmetric-folded W0: slot (c, p) holds pair (i_of[c,p], p%32)
    i_of = _sym_pack()
    Wsym = W0.reshape(K, F0, F0) + W0.reshape(K, F0, F0).transpose(0, 2, 1)
    Wsym[:, np.arange(F0), np.arange(F0)] = W0[:, np.arange(F0), np.arange(F0)]
    w0sym = np.zeros((128, NC0 * K), np.float16)
    p_ = np.arange(128)
    for c in range(NC0):
        live = i_of[c] >= 0
        ii = np.where(live, i_of[c], 0)
        jj = p_ % F0
        w0sym[:, c * K : (c + 1) * K] = np.where(
            live[:, None], Wsym[:, ii, jj].T, 0.0
        ).astype(np.float16)

    # L1 kh0 (h-half) fp16: [i, j*128 + m] = W1[m, i, j]
    w1h16 = np.ascontiguousarray(
        W1[:128].transpose(1, 2, 0).reshape(H, F0 * 128)
    ).astype(np.float16)
    # L1 kh1 fp8 1-pass: [i, (g*2+pair)*128 + m] = q8(W1[128+m, i, 2g+pair])
    w1r = W1[128:].transpose(1, 2, 0)            # [i, j, m]
    w1r8 = np.clip(w1r, -240, 240).astype(E4).reshape(H, F0 * 128)
    # fp16 copy for the kh1 f16 prefix quads
    w1r16 = np.ascontiguousarray(w1r.reshape(H, F0 * 128)).astype(np.float16)

    # L2 both kh fp8 2-pass: [i, ((g*2+kh)*2+pair)*128 + m] = W2[kh*128+m, i, 2g+pair]
    w2 = W2.transpose(1, 2, 0).reshape(H, NG, 2, 2, 128)  # [i, g, pair, kh, m]
    w2 = np.ascontiguousarray(w2.transpose(0, 1, 3, 2, 4))  # [i, g, kh, pair, m]
    w2hi = np.clip(w2, -240, 240).astype(E4)
    w2lo = (w2 - w2hi.astype(np.float32)).astype(E5)
    w2hi8 = w2hi.reshape(H, NG * 2 * 2 * 128)
    w2lo8 = w2lo.reshape(H, NG * 2 * 2 * 128)

    # fp16 W2 for inline-f16 L2 quads: [i, j*256 + kh*128 + m] = W2[kh*128+m, i, j]
    w2f16 = np.ascontiguousarray(
        W2.transpose(1, 2, 0).reshape(H, F0 * K)
    ).astype(np.float16)

    b0t = np.ascontiguousarray(b0.reshape(2, 128).T).astype(np.float32)
    b1t = np.ascontiguousarray(b1.reshape(2, 128).T).astype(np.float32)
    b2t = np.ascontiguousarray(b2.reshape(2, 128).T).astype(np.float32)

    ii_all = np.where(i_of >= 0, i_of, (p_ % F0)[None, :])  # [NC0, 128]

    in_maps = []
    for c in range(NCORES):
        fc = feat[c * BL : (c + 1) * BL]                        # [64, 32, 32]
        featT = np.ascontiguousarray(fc.transpose(1, 0, 2)).reshape(F0, BD)
        featT = featT.astype(np.float16)
        featR = np.ascontiguousarray(featT[p_ % F0])            # [128, BD]
        # t-major featHsym: [p, t*(NC0*NT) + cc*NT + n] = featT[i_of[cc,p], t*NT+n]
        fHs = np.concatenate(
            [
                featT[ii_all[cc], t * NT : (t + 1) * NT]
                for t in range(BD // NT)
                for cc in range(NC0)
            ],
            axis=1,
        )                                                        # [128, NC0*BD]
        in_maps.append(
            {
                "featT16": featT,
                "featR": featR,
                "featHsym": np.ascontiguousarray(fHs),
                "w0sym": w0sym,
                "w1h16": w1h16,
                "w1r8": w1r8.view(np.uint8),
                "w1r16": w1r16,
                "w2hi8": w2hi8.view(np.uint8),
                "w2lo8": w2lo8.view(np.uint8),
                "w2f16": w2f16,
                "b0t": b0t,
                "b1t": b1t,
                "b2t": b2t,
            }
        )
    return in_maps


def kernel(feat, W0, b0, W1, b1, W2, b2):
    global LAST_RESULTS
    if "nc" not in _CACHE:
        _CACHE["nc"] = _build_program()
    nc = _CACHE["nc"]
    in_maps = _host_prep(feat, W0, b0, W1, b1, W2, b2)
    res = run_bass_kernel_spmd(nc, in_maps, core_ids=list(range(NCORES)))
    LAST_RESULTS = res
    out = np.concatenate([res.results[c]["out"].T for c in range(NCORES)], axis=0)
    return np.ascontiguousarray(out, dtype=np.float32)
